# revision 5
# baseline (speedup 1.0000x reference)
"""Multi-head self-attention with RoPE on 8 Trainium2 NeuronCores.

Transfer-minimal fp16 design.  Core c (b=c//2, g=c%2) handles batch b and
heads [8g, 8g+8).  The host sends each input byte exactly once (fp16):
  - xh   [1024,1024]  x[b] rows g*1024..+1024, PRE-TRANSPOSED to [C, N/2]
  - wkv4 [1, 262144]  flat [K-slice | V-slice], Wkv rows 256b..+256 of the
                      g-half columns (own AllGather so V-proj starts early)
  - pack [1, 393216]  flat wq (rows 256b, cols 512g) | wo (rows 512g+128b)
                      | cs (1/4 of the stacked cos/sin tables)
  - rt, bias          rotate-half permutation matrix, b_out/2
On device: a pair AllGather rebuilds x[b]^T (static placement), two
group-of-4 AllGathers (cores sharing g) rebuild the g-half weight slices
and tables at static addresses.  V (fused [v_h | ones]) stays in SBUF.
Sim matmuls use per-head zero-padded K tiles (kTz) so the contraction is
a full 128 rows.  All matmuls fp16 with fp32 PSUM accumulation; exp on
the scalar engine feeds a pipelined P@V whose ones-columns produce the
softmax denominators.  Output projection interleaves per query-chunk with
pair ReduceScatters (fp16); each core returns 1024 rows (256-row
interleave) that the host reassembles and upcasts to fp32.
"""

import numpy as np

import concourse.mybir as mybir
import concourse.tile as tile
from concourse import bacc
from concourse.bass_utils import run_bass_kernel_spmd

B, N, H, DH = 4, 2048, 16, 64
C = H * DH            # 1024
HG = H // 2           # 8 heads per core
CG = HG * DH          # 512 channels per core
NCORES = 8
ROPE_BASE = 10000.0

F16 = mybir.dt.float16
F32 = mybir.dt.float32

KC = C // 128         # 8 contraction chunks over C
MT = CG // 128        # 4 m-tiles of per-core q/k channels (2 heads each)
NQ = N // 512         # 4 query column chunks
NKT = N // 128        # 16 key/seq row tiles

PAIRS = [[0, 1], [2, 3], [4, 5], [6, 7]]
GGRP = [[0, 2, 4, 6], [1, 3, 5, 7]]
EXP = mybir.ActivationFunctionType.Exp
SCALE = float(1.0 / np.sqrt(DH))

# weight slabs (per group-of-4 member), flat f16.  wkv is its own AG (first
# weight needed); wq | wo | cs ride a second AG.
PK_WKV = 256 * 2 * CG       # 262144
PK_WQ = 256 * CG            # 131072
PK_WO = 128 * C             # 131072
PK_CS = 64 * N              # 131072
PK = PK_WQ + PK_WO + PK_CS
PK_OFF = (0, PK_WQ, PK_WQ + PK_WO)


def _build(mode="full", reps=1):
    """mode: "full" | "noccl" (skip output RS, dump partial) | "proj"
    (projections only, dump qT/kT/v)."""
    nc = bacc.Bacc("TRN2", target_bir_lowering=False, num_devices=NCORES)

    xh_e = nc.declare_dram_parameter("xh", [N // 2, C], F16, isOutput=False)
    wkv_e = nc.declare_dram_parameter("wkv4", [1, PK_WKV], F16, isOutput=False)
    pk_e = nc.declare_dram_parameter("pack", [1, PK], F16, isOutput=False)
    rt_e = nc.declare_dram_parameter("rt", [128, 128], F16, isOutput=False)
    bias_e = nc.declare_dram_parameter("bias", [1, C], F16, isOutput=False)
    if mode == "full":
        out_e = nc.declare_dram_parameter("out", [N // 2, C], F16, isOutput=True)
    elif mode == "noccl":
        part_e = nc.declare_dram_parameter("part", [N, C], F16, isOutput=True)
    elif mode == "proj":
        dq_e = nc.declare_dram_parameter("dbg_q", [CG, N], F32, isOutput=True)
        dk_e = nc.declare_dram_parameter("dbg_k", [CG, N], F32, isOutput=True)
        dv_e = nc.declare_dram_parameter("dbg_v", [NKT * 128, HG * 128], F32, isOutput=True)

    with tile.TileContext(nc) as tc:
        for _rep in range(reps):
            _sfx = f"_{_rep}" if reps > 1 else ""
            with tc.tile_pool(name="persist" + _sfx, bufs=1) as p_pers, \
                 tc.tile_pool(name="dram" + _sfx, bufs=1, space="DRAM") as p_dram:
                # ---------- DRAM scratch + collectives ----------
                wkv_d0 = p_dram.tile([4, PK_WKV], F16, name="wkv_d")
                xg_d = p_dram.tile([N, C], F16, name="xg_d")
                pk_d = p_dram.tile([4, PK], F16, name="pk_d")
                part_d = p_dram.tile([N, C], F16, name="part_d")
                part3 = part_d.rearrange("(s p) c -> s p c", p=128)
                rs_ch = [p_dram.tile([N // 8, C], F16, name=f"rs{q}")
                         for q in range(4)]

                # collectives may not read IO tensors: stage d2d first
                xh_s = p_dram.tile([N // 2, C], F16, name="xh_s")
                nc.sync.dma_start(out=xh_s, in_=xh_e.ap())
                wkv_s = p_dram.tile([1, PK_WKV], F16, name="wkv_s")
                nc.sync.dma_start(out=wkv_s, in_=wkv_e.ap())
                pk_s = p_dram.tile([1, PK], F16, name="pk_s")
                nc.sync.dma_start(out=pk_s, in_=pk_e.ap())
                wkv_d = wkv_d0
                nc.gpsimd.collective_compute(
                    "AllGather", mybir.AluOpType.bypass, replica_groups=GGRP,
                    ins=[wkv_s[:]], outs=[wkv_d[:]])
                nc.gpsimd.collective_compute(
                    "AllGather", mybir.AluOpType.bypass, replica_groups=PAIRS,
                    ins=[xh_s[:]], outs=[xg_d[:]])
                nc.gpsimd.collective_compute(
                    "AllGather", mybir.AluOpType.bypass, replica_groups=GGRP,
                    ins=[pk_s[:]], outs=[pk_d[:]])

                # per-rank views into the gathered slabs
                wkv_v = wkv_d.rearrange("b (r m) -> b r m", m=2 * CG)  # [4,256,1024]
                wq_v = pk_d[:, PK_OFF[0]:PK_OFF[0] + PK_WQ].rearrange(
                    "b (r m) -> b r m", m=CG)          # [4, 256, 512]
                wo_v = pk_d[:, PK_OFF[1]:PK_OFF[1] + PK_WO].rearrange(
                    "b (r m) -> b r m", m=C)           # [4, 128, 1024]
                cs_v = pk_d[:, PK_OFF[2]:PK_OFF[2] + PK_CS].rearrange(
                    "b (r m) -> b r m", m=N)           # [4, 64, 2048]

                # ---------- small constants ----------
                ones1_r = p_pers.tile([1, 128], F16, name="ones1_r")
                nc.vector.memset(ones1_r, 1.0)
                rt_s = p_pers.tile([128, 128], F16, name="rt_s")
                nc.sync.dma_start(out=rt_s, in_=rt_e.ap())
                bias_r = p_pers.tile([1, C], F16, name="bias_r")
                nc.sync.dma_start(out=bias_r, in_=bias_e.ap())

                # V (+ones) stationary tiles, persist through attention
                vsb = [p_pers.tile([128, HG, 128], F16, name=f"vsb{s}")
                       for s in range(NKT)]
                for s in range(NKT):
                    nc.vector.memset(vsb[s][:, :, 64:128], 1.0)

                with tc.tile_pool(name="qk" + _sfx, bufs=1) as p_qk:
                    qT = [p_qk.tile([128, N], F16, name=f"qT{m}") for m in range(MT)]
                    # per-head K tiles, zero-padded on the complementary dh
                    # rows so sim matmuls run full 128-row contraction:
                    # kTz[2m]   = [k_head(0:64) | 0]   (even head of pair m)
                    # kTz[2m+1] = [0 | k_head(64:128)] (odd head stays put)
                    kTz = [p_qk.tile([128, N], F16, name=f"kTz{h}") for h in range(H // 2)]
                    for m in range(MT):
                        nc.vector.memset(kTz[2 * m][64:128, :], 0.0)
                        nc.vector.memset(kTz[2 * m + 1][0:64, :], 0.0)

                    # ---------- xT + projections ----------
                    # xh arrives pre-transposed [C, N/2]; the pair AG stacks
                    # [own-half-T | pair-half-T] so xT tiles are plain loads.
                    with tc.tile_pool(name="xt" + _sfx, bufs=1) as p_xt:
                        xT = [p_xt.tile([128, N], F16, name=f"xT{c}") for c in range(KC)]
                        # [2 half, 8 cb, 128 p, 1024 n] view of the gathered x;
                        # loads ride the Activation hwdge queue (idle here) so
                        # they overlap the wv loads on the SP queue.
                        xg_v = xg_d.rearrange("(h q p) n -> q p h n", h=2, p=128)
                        for cb in range(KC):
                            nc.scalar.dma_start(
                                out=xT[cb].rearrange("p (h n) -> p h n", h=2),
                                in_=xg_v[cb])

                        with tc.tile_pool(name="wvp" + _sfx, bufs=1) as p_wv, \
                             tc.tile_pool(name="psV" + _sfx, bufs=4, space="PSUM") as pp_v:
                            wv_r = [p_wv.tile([128, CG], F16, name=f"wv{c}") for c in range(KC)]
                            for c in range(KC):
                                rsl = slice(128 * (c % 2), 128 * (c % 2) + 128)
                                nc.sync.dma_start(
                                    out=wv_r[c], in_=wkv_v[c // 2, rsl, CG:2 * CG])
                            for s in range(NKT):
                                ps = pp_v.tile([128, CG], F32, name=f"pv{s}", tag="pv")
                                for c in range(KC):
                                    nc.tensor.matmul(
                                        ps, xT[c][:, s * 128:(s + 1) * 128], wv_r[c],
                                        start=(c == 0), stop=(c == KC - 1))
                                nc.vector.tensor_copy(
                                    vsb[s][:, :, 0:64],
                                    ps.rearrange("p (h d) -> p h d", d=DH))

                        # q/k projections + RoPE
                        with tc.tile_pool(name="qkw" + _sfx, bufs=1) as p_qkw, \
                             tc.tile_pool(name="stage_a", bufs=2) as p_sta, \
                             tc.tile_pool(name="psA" + _sfx, bufs=4, space="PSUM") as pp_a, \
                             tc.tile_pool(name="psR" + _sfx, bufs=2, space="PSUM") as pp_r:
                            wq_r = [p_qkw.tile([128, CG], F16, name=f"wq{c}") for c in range(KC)]
                            wk_r = [p_qkw.tile([128, CG], F16, name=f"wk{c}") for c in range(KC)]
                            for c in range(KC):
                                rsl = slice(128 * (c % 2), 128 * (c % 2) + 128)
                                nc.sync.dma_start(out=wq_r[c], in_=wq_v[c // 2, rsl])
                                nc.sync.dma_start(out=wk_r[c], in_=wkv_v[c // 2, rsl, 0:CG])
                            cosf = p_qkw.tile([128, N], F16, name="cosf")
                            nc.sync.dma_start(out=cosf[0:64], in_=cs_v[0])
                            nc.sync.dma_start(out=cosf[64:128], in_=cs_v[1])
                            sinf = p_qkw.tile([128, N], F16, name="sinf")
                            nc.sync.dma_start(out=sinf[0:64], in_=cs_v[2])
                            nc.sync.dma_start(out=sinf[64:128], in_=cs_v[3])

                            def _finish_rope(pend):
                                dst, m, n, qsb, lbl = pend
                                ns = slice(n * 512, (n + 1) * 512)
                                pr = pp_r.tile([128, 512], F32, name=f"pr{lbl}{m}{n}", tag="pr")
                                nc.tensor.matmul(pr, rt_s, qsb, start=True, stop=True)
                                t1 = p_sta.tile([128, 512], F16, name=f"t1{lbl}{m}{n}",
                                                tag="t1", bufs=2)
                                nc.vector.tensor_mul(t1, qsb, cosf[:, ns])
                                t2 = p_sta.tile([128, 512], F16, name=f"t2{lbl}{m}{n}",
                                                tag="t2", bufs=2)
                                nc.vector.tensor_mul(t2, pr, sinf[:, ns])
                                if lbl == "q":
                                    nc.vector.tensor_add(dst[m][:, ns], t1, t2)
                                else:
                                    nc.vector.tensor_add(
                                        kTz[2 * m][0:64, ns], t1[0:64], t2[0:64])
                                    nc.vector.tensor_add(
                                        kTz[2 * m + 1][64:128, ns], t1[64:128], t2[64:128])

                            pend = None
                            for m in range(MT):
                                for lbl, w_r, dst in (("q", wq_r, qT), ("k", wk_r, None)):
                                    for n in range(NQ):
                                        ns = slice(n * 512, (n + 1) * 512)
                                        ps = pp_a.tile([128, 512], F32,
                                                       name=f"ps{lbl}{m}{n}", tag="ps")
                                        for c in range(KC):
                                            nc.tensor.matmul(
                                                ps, w_r[c][:, m * 128:(m + 1) * 128],
                                                xT[c][:, ns],
                                                start=(c == 0), stop=(c == KC - 1))
                                        qsb = p_sta.tile([128, 512], F16,
                                                         name=f"qsb{lbl}{m}{n}",
                                                         tag="qsb", bufs=3)
                                        nc.vector.tensor_copy(qsb, ps)
                                        if pend is not None:
                                            _finish_rope(pend)
                                        pend = (dst, m, n, qsb, lbl)
                            _finish_rope(pend)

                    if mode == "proj":
                        with tc.tile_pool(name="dbg" + _sfx, bufs=2) as p_dbg:
                            for m in range(MT):
                                for lbl, dst_e in (("q", dq_e), ("k", dk_e)):
                                    db = p_dbg.tile([128, N], F32, name=f"db{lbl}{m}", tag="db")
                                    if lbl == "q":
                                        nc.vector.tensor_copy(db, qT[m])
                                    else:
                                        nc.vector.tensor_copy(db[0:64], kTz[2 * m][0:64])
                                        nc.vector.tensor_copy(db[64:128], kTz[2 * m + 1][64:128])
                                    nc.sync.dma_start(
                                        out=dst_e.ap().rearrange("(m p) n -> m p n", p=128)[m],
                                        in_=db)
                            dv3 = dv_e.ap().rearrange("(s p) c -> s p c", p=128)
                            for s in range(NKT):
                                vxf = p_dbg.tile([128, HG * 128], F32, name=f"dvf{s}", tag="dvf")
                                nc.vector.tensor_copy(
                                    vxf, vsb[s].rearrange("p h d -> p (h d)"))
                                nc.sync.dma_start(out=dv3[s], in_=vxf)
                        attn_enabled = False
                    else:
                        attn_enabled = True

                    # ---------- attention + output projection ----------
                    if attn_enabled:
                      with tc.tile_pool(name="oTp" + _sfx, bufs=1) as p_oT, \
                         tc.tile_pool(name="attn" + _sfx, bufs=1) as p_at, \
                         tc.tile_pool(name="wop" + _sfx, bufs=1) as p_wo, \
                         tc.tile_pool(name="psS" + _sfx, bufs=2, space="PSUM") as pp_s, \
                         tc.tile_pool(name="psO" + _sfx, bufs=4, space="PSUM") as pp_o:
                          oT = [p_oT.tile([128, N], F16, name=f"oT{m}") for m in range(MT)]
                          wo_r = [p_wo.tile([128, C], F16, name=f"wo{c}") for c in range(MT)]
                          for c in range(MT):
                              nc.sync.dma_start(out=wo_r[c], in_=wo_v[c])
                          # bias replicated across partitions once (PE outer
                          # product), so outproj adds it on the DVE copy
                          bias128 = p_wo.tile([128, C], F16, name="bias128")
                          for half in range(2):
                              osl = slice(half * 512, (half + 1) * 512)
                              ps_b = pp_o.tile([128, 512], F32,
                                               name=f"psb{half}", tag="pso")
                              nc.tensor.matmul(ps_b, ones1_r, bias_r[:, osl],
                                               start=True, stop=True)
                              nc.vector.tensor_copy(bias128[:, osl], ps_b)

                          # exp groups: 16 kc chunks -> 8 pairs (fits 4 PSUM
                          # banks for sim, leaving 4 for pso/outproj slack)
                          GRP = [(2 * i, 2 * i + 2) for i in range(8)]

                          def _emit_pv(pend_pv, pso, hp):
                              (k0, k1), exs = pend_pv
                              for half in range(2):
                                  h = hp * 2 + half
                                  for j in range(k1 - k0):
                                      kc = k0 + j
                                      nc.tensor.matmul(
                                          pso[half], vsb[kc][:, h, :], exs[half][:, j],
                                          start=(kc == 0), stop=(kc == NKT - 1))

                          def _emit_outproj(s):
                              for half in range(2):
                                  osl = slice(half * 512, (half + 1) * 512)
                                  ps = pp_o.tile([128, 512], F32, name=f"po{s}{half}",
                                                 tag="pso")
                                  for cc in range(MT):
                                      nc.tensor.matmul(
                                          ps, oT[cc][:, s * 128:(s + 1) * 128],
                                          wo_r[cc][:, osl],
                                          start=(cc == 0), stop=(cc == MT - 1))
                                  ob = p_at.tile([128, 512], F16, name=f"ob{s}{half}",
                                                 tag="ob", bufs=4)
                                  nc.vector.tensor_add(ob, ps, bias128[:, osl])
                                  nc.sync.dma_start(out=part3[s][:, osl], in_=ob)
                              if s % 4 == 3 and mode == "full":
                                  q = s // 4
                                  nc.gpsimd.collective_compute(
                                      "ReduceScatter", mybir.AluOpType.add,
                                      replica_groups=PAIRS,
                                      ins=[part_d[q * 512:(q + 1) * 512]],
                                      outs=[rs_ch[q][:]])
                                  nc.sync.dma_start(
                                      out=out_e.ap()[q * 256:(q + 1) * 256],
                                      in_=rs_ch[q][:])

                          for qc in range(NQ):
                              qs = slice(qc * 512, (qc + 1) * 512)
                              for hp in range(MT):
                                  pso = [
                                      pp_o.tile([128, 512], F32, name=f"pso{qc}{hp}{h}",
                                                tag="pso")
                                      for h in range(2)
                                  ]
                                  pend_pv = None
                                  for (k0, k1) in GRP:
                                      exs = []
                                      for half in range(2):
                                          sim = pp_s.tile([128, 2, 512], F32,
                                                          name=f"sim{qc}{hp}{k0}{half}",
                                                          tag="sim")
                                          for j in range(k1 - k0):
                                              kc = k0 + j
                                              nc.tensor.matmul(
                                                  sim[:, j],
                                                  kTz[2 * hp + half][:, kc * 128:(kc + 1) * 128],
                                                  qT[hp][:, qs],
                                                  start=True, stop=True)
                                          ex = p_at.tile([128, 2, 512], F16,
                                                         name=f"ex{qc}{hp}{k0}{half}",
                                                         tag="ex", bufs=6)
                                          nc.scalar.activation(
                                              ex[:, 0:k1 - k0], sim[:, 0:k1 - k0],
                                              EXP, scale=SCALE)
                                          exs.append(ex)
                                      if pend_pv is not None:
                                          _emit_pv(pend_pv, pso, hp)
                                      pend_pv = ((k0, k1), exs)
                                  _emit_pv(pend_pv, pso, hp)

                                  for half in range(2):
                                      rc = p_at.tile([64, 512], F32,
                                                     name=f"rc{qc}{hp}{half}", tag="rc", bufs=4)
                                      nc.vector.reciprocal(rc, pso[half][64:128])
                                      nc.vector.tensor_mul(
                                          oT[hp][half * 64:(half + 1) * 64, qs],
                                          pso[half][0:64], rc)

                              # all head pairs done for this qc: project + reduce
                              for s in range(qc * 4, qc * 4 + 4):
                                  _emit_outproj(s)
                          if mode != "full":
                              nc.sync.dma_start(out=part_e.ap(), in_=part_d[:])

    nc.compile()
    return nc


_NC = {}


def _get_nc(mode="full", reps=1):
    key = (mode, reps)
    if key not in _NC:
        _NC[key] = _build(mode, reps)
    return _NC[key]


def _rope_tables():
    inv = (1.0 / (ROPE_BASE ** (np.arange(0, DH, 2, dtype=np.float32) / DH))).astype(np.float32)
    t = np.arange(N, dtype=np.float32)
    freqs = np.outer(t, inv).astype(np.float32)           # [N, 32]
    emb = np.concatenate([freqs, freqs], axis=-1)         # [N, 64]
    cosT = np.cos(emb).astype(np.float32).T               # [64, N]
    sinT = np.sin(emb).astype(np.float32).T
    cosF = np.ascontiguousarray(np.tile(cosT, (2, 1)))    # [128, N]
    sinF = np.ascontiguousarray(np.tile(sinT, (2, 1)))
    return cosF, sinF


def _rot_matrix():
    # rotate_half as a left-multiply in [d, n] layout: rot = R @ q
    R = np.zeros((DH, DH), np.float32)
    half = DH // 2
    for d in range(half):
        R[d, d + half] = -1.0
        R[d + half, d] = 1.0
    Rbig = np.zeros((128, 128), np.float32)
    Rbig[:DH, :DH] = R
    Rbig[DH:, DH:] = R
    return np.ascontiguousarray(Rbig.T)  # lhsT for out = Rbig @ rhs


_CONST = None


def _consts():
    global _CONST
    if _CONST is None:
        cosF, sinF = _rope_tables()
        CS = np.concatenate([cosF, sinF], axis=0).astype(np.float16)  # [256, N]
        rt16 = _rot_matrix().astype(np.float16)
        _CONST = (CS, rt16)
    return _CONST


def kernel(x, Wq, Wkv, Wout, b_out):
    f16 = np.float16
    x = np.asarray(x)
    Wq = np.asarray(Wq)
    Wkv = np.asarray(Wkv)
    Wout = np.asarray(Wout)
    b_out = np.asarray(b_out)

    CS, rt16 = _consts()
    bias16 = (b_out.astype(np.float32) * 0.5).reshape(1, C).astype(f16)

    in_maps = []
    for core in range(NCORES):
        b, g = core // 2, core % 2
        gs = slice(CG * g, CG * (g + 1))
        vs = slice(C + CG * g, C + CG * (g + 1))
        rs = slice(256 * b, 256 * (b + 1))
        wkv4 = np.empty((1, PK_WKV), f16)
        pkv = wkv4.reshape(256, 2 * CG)
        pkv[:, 0:CG] = Wkv[rs, gs].astype(f16)
        pkv[:, CG:2 * CG] = Wkv[rs, vs].astype(f16)
        pack = np.empty((1, PK), f16)
        pack[0, PK_OFF[0]:PK_OFF[0] + PK_WQ] = Wq[rs, gs].astype(f16).reshape(-1)
        pack[0, PK_OFF[1]:PK_OFF[1] + PK_WO] = \
            Wout[CG * g + 128 * b:CG * g + 128 * (b + 1), :].astype(f16).reshape(-1)
        pack[0, PK_OFF[2]:PK_OFF[2] + PK_CS] = CS[64 * b:64 * (b + 1)].reshape(-1)
        in_maps.append({
            "xh": x[b, N // 2 * g:N // 2 * (g + 1)].T.astype(f16),
            "wkv4": wkv4,
            "pack": pack,
            "rt": rt16,
            "bias": bias16,
        })

    res = run_bass_kernel_spmd(_get_nc(), in_maps, core_ids=list(range(NCORES)))
    out = np.empty((B, N, C), np.float32)
    for b in range(B):
        e = res.results[2 * b]["out"].astype(np.float32)
        o = res.results[2 * b + 1]["out"].astype(np.float32)
        for q in range(4):
            out[b, 512 * q:512 * q + 256] = e[256 * q:256 * (q + 1)]
            out[b, 512 * q + 256:512 * (q + 1)] = o[256 * q:256 * (q + 1)]
    return out


# revision 7
# speedup vs baseline: 1.0287x; 1.0287x over previous
"""Multi-head self-attention with RoPE on 8 Trainium2 NeuronCores.

Transfer-minimal fp16 design.  Core c (b=c//2, g=c%2) handles batch b and
heads [8g, 8g+8).  The host sends each input byte exactly once (fp16):
  - xh   [1024,1024]  x[b] rows g*1024..+1024, PRE-TRANSPOSED to [C, N/2]
  - wkv4 [1, 262144]  flat [K-slice | V-slice], Wkv rows 256b..+256 of the
                      g-half columns (own AllGather so V-proj starts early)
  - pack [1, 393216]  flat wq (rows 256b, cols 512g) | wo (rows 512g+128b)
                      | cs (1/4 of the stacked cos/sin tables)
  - rt, bias          rotate-half permutation matrix, b_out/2
On device: a pair AllGather rebuilds x[b]^T (static placement), two
group-of-4 AllGathers (cores sharing g) rebuild the g-half weight slices
and tables at static addresses.  V (fused [v_h | ones]) stays in SBUF.
Sim matmuls use per-head zero-padded K tiles (kTz) so the contraction is
a full 128 rows.  All matmuls fp16 with fp32 PSUM accumulation; exp on
the scalar engine feeds a pipelined P@V whose ones-columns produce the
softmax denominators.  Output projection interleaves per query-chunk with
pair ReduceScatters (fp16); each core returns 1024 rows (256-row
interleave) that the host reassembles and upcasts to fp32.
"""

import numpy as np

import concourse.mybir as mybir
import concourse.tile as tile
from concourse import bacc
from concourse.bass_utils import run_bass_kernel_spmd

B, N, H, DH = 4, 2048, 16, 64
C = H * DH            # 1024
HG = H // 2           # 8 heads per core
CG = HG * DH          # 512 channels per core
NCORES = 8
ROPE_BASE = 10000.0

F16 = mybir.dt.float16
F32 = mybir.dt.float32

KC = C // 128         # 8 contraction chunks over C
MT = CG // 128        # 4 m-tiles of per-core q/k channels (2 heads each)
NQ = N // 512         # 4 query column chunks
NKT = N // 128        # 16 key/seq row tiles

PAIRS = [[0, 1], [2, 3], [4, 5], [6, 7]]
GGRP = [[0, 2, 4, 6], [1, 3, 5, 7]]
EXP = mybir.ActivationFunctionType.Exp
SCALE = float(1.0 / np.sqrt(DH))

# weight slabs (per group-of-4 member), flat f16.  wkv is its own AG (first
# weight needed); wq | wo | cs ride a second AG.
PK_WKV = 256 * 2 * CG       # 262144
PK_WQ = 256 * CG            # 131072
PK_WO = 128 * C             # 131072
PK_CS = 64 * N              # 131072
PK = PK_WQ + PK_WO + PK_CS
PK_OFF = (0, PK_WQ, PK_WQ + PK_WO)


def _build(mode="full", reps=1):
    """mode: "full" | "noccl" (skip output RS, dump partial) | "proj"
    (projections only, dump qT/kT/v)."""
    nc = bacc.Bacc("TRN2", target_bir_lowering=False, num_devices=NCORES)

    xh_e = nc.declare_dram_parameter("xh", [N // 2, C], F16, isOutput=False)
    wkv_e = nc.declare_dram_parameter("wkv4", [1, PK_WKV], F16, isOutput=False)
    pk_e = nc.declare_dram_parameter("pack", [1, PK], F16, isOutput=False)
    rt_e = nc.declare_dram_parameter("rt", [128, 128], F16, isOutput=False)
    bias_e = nc.declare_dram_parameter("bias", [1, C], F16, isOutput=False)
    if mode == "full":
        out_e = nc.declare_dram_parameter("out", [N // 2, C], F16, isOutput=True)
    elif mode == "noccl":
        part_e = nc.declare_dram_parameter("part", [N, C], F16, isOutput=True)
    elif mode == "proj":
        dq_e = nc.declare_dram_parameter("dbg_q", [CG, N], F32, isOutput=True)
        dk_e = nc.declare_dram_parameter("dbg_k", [CG, N], F32, isOutput=True)
        dv_e = nc.declare_dram_parameter("dbg_v", [NKT * 128, HG * 128], F32, isOutput=True)

    with tile.TileContext(nc) as tc:
        for _rep in range(reps):
            _sfx = f"_{_rep}" if reps > 1 else ""
            with tc.tile_pool(name="persist" + _sfx, bufs=1) as p_pers, \
                 tc.tile_pool(name="dram" + _sfx, bufs=1, space="DRAM") as p_dram:
                # ---------- DRAM scratch + collectives ----------
                wkv_d0 = p_dram.tile([4, PK_WKV], F16, name="wkv_d")
                xg_d = p_dram.tile([N, C], F16, name="xg_d")
                pk_d = p_dram.tile([4, PK], F16, name="pk_d")
                part_d = p_dram.tile([N, C], F16, name="part_d")
                part3 = part_d.rearrange("(s p) c -> s p c", p=128)
                rs_ch = [p_dram.tile([N // 8, C], F16, name=f"rs{q}")
                         for q in range(4)]

                # collectives may not read IO tensors: stage d2d first
                xh_s = p_dram.tile([N // 2, C], F16, name="xh_s")
                nc.sync.dma_start(out=xh_s, in_=xh_e.ap())
                wkv_s = p_dram.tile([1, PK_WKV], F16, name="wkv_s")
                nc.sync.dma_start(out=wkv_s, in_=wkv_e.ap())
                pk_s = p_dram.tile([1, PK], F16, name="pk_s")
                nc.sync.dma_start(out=pk_s, in_=pk_e.ap())
                wkv_d = wkv_d0
                # x AG split in two C-row halves: the first unblocks V-proj
                # accumulation over kc 0..3 while the rest still transfers
                xgA = p_dram.tile([N // 2, C], F16, name="xgA")
                xgB = p_dram.tile([N // 2, C], F16, name="xgB")
                nc.gpsimd.collective_compute(
                    "AllGather", mybir.AluOpType.bypass, replica_groups=PAIRS,
                    ins=[xh_s[0:512]], outs=[xgA[:]])
                nc.gpsimd.collective_compute(
                    "AllGather", mybir.AluOpType.bypass, replica_groups=GGRP,
                    ins=[wkv_s[:]], outs=[wkv_d[:]])
                nc.gpsimd.collective_compute(
                    "AllGather", mybir.AluOpType.bypass, replica_groups=PAIRS,
                    ins=[xh_s[512:1024]], outs=[xgB[:]])
                nc.gpsimd.collective_compute(
                    "AllGather", mybir.AluOpType.bypass, replica_groups=GGRP,
                    ins=[pk_s[:]], outs=[pk_d[:]])

                # per-rank views into the gathered slabs
                wkv_v = wkv_d.rearrange("b (r m) -> b r m", m=2 * CG)  # [4,256,1024]
                wq_v = pk_d[:, PK_OFF[0]:PK_OFF[0] + PK_WQ].rearrange(
                    "b (r m) -> b r m", m=CG)          # [4, 256, 512]
                wo_v = pk_d[:, PK_OFF[1]:PK_OFF[1] + PK_WO].rearrange(
                    "b (r m) -> b r m", m=C)           # [4, 128, 1024]
                cs_v = pk_d[:, PK_OFF[2]:PK_OFF[2] + PK_CS].rearrange(
                    "b (r m) -> b r m", m=N)           # [4, 64, 2048]

                # ---------- small constants ----------
                ones1_r = p_pers.tile([1, 128], F16, name="ones1_r")
                nc.vector.memset(ones1_r, 1.0)
                rt_s = p_pers.tile([128, 128], F16, name="rt_s")
                nc.sync.dma_start(out=rt_s, in_=rt_e.ap())
                bias_r = p_pers.tile([1, C], F16, name="bias_r")
                nc.sync.dma_start(out=bias_r, in_=bias_e.ap())

                # V (+ones) stationary tiles, persist through attention
                vsb = [p_pers.tile([128, HG, 128], F16, name=f"vsb{s}")
                       for s in range(NKT)]
                for s in range(NKT):
                    nc.vector.memset(vsb[s][:, :, 64:128], 1.0)

                with tc.tile_pool(name="qk" + _sfx, bufs=1) as p_qk:
                    qT = [p_qk.tile([128, N], F16, name=f"qT{m}") for m in range(MT)]
                    # per-head K tiles, zero-padded on the complementary dh
                    # rows so sim matmuls run full 128-row contraction:
                    # kTz[2m]   = [k_head(0:64) | 0]   (even head of pair m)
                    # kTz[2m+1] = [0 | k_head(64:128)] (odd head stays put)
                    kTz = [p_qk.tile([128, N], F16, name=f"kTz{h}") for h in range(H // 2)]
                    for m in range(MT):
                        nc.vector.memset(kTz[2 * m][64:128, :], 0.0)
                        nc.vector.memset(kTz[2 * m + 1][0:64, :], 0.0)

                    # ---------- xT + projections ----------
                    # xh arrives pre-transposed [C, N/2]; the pair AG stacks
                    # [own-half-T | pair-half-T] so xT tiles are plain loads.
                    with tc.tile_pool(name="xt" + _sfx, bufs=1) as p_xt:
                        xT = [p_xt.tile([128, N], F16, name=f"xT{c}") for c in range(KC)]
                        # [2 half, 8 cb, 128 p, 1024 n] view of the gathered x;
                        # loads ride the Activation hwdge queue (idle here) so
                        # they overlap the wv loads on the SP queue.
                        xgA_v = xgA.rearrange("(h q p) n -> q p h n", h=2, p=128)
                        xgB_v = xgB.rearrange("(h q p) n -> q p h n", h=2, p=128)
                        for cb in range(KC):
                            src = xgA_v[cb] if cb < 4 else xgB_v[cb - 4]
                            nc.scalar.dma_start(
                                out=xT[cb].rearrange("p (h n) -> p h n", h=2),
                                in_=src)

                        with tc.tile_pool(name="wvp" + _sfx, bufs=1) as p_wv, \
                             tc.tile_pool(name="psV" + _sfx, bufs=4, space="PSUM") as pp_v:
                            wv_r = [p_wv.tile([128, CG], F16, name=f"wv{c}") for c in range(KC)]
                            for c in range(KC):
                                rsl = slice(128 * (c % 2), 128 * (c % 2) + 128)
                                nc.sync.dma_start(
                                    out=wv_r[c], in_=wkv_v[c // 2, rsl, CG:2 * CG])
                            for s in range(NKT):
                                ps = pp_v.tile([128, CG], F32, name=f"pv{s}", tag="pv")
                                for c in range(KC):
                                    nc.tensor.matmul(
                                        ps, xT[c][:, s * 128:(s + 1) * 128], wv_r[c],
                                        start=(c == 0), stop=(c == KC - 1))
                                nc.vector.tensor_copy(
                                    vsb[s][:, :, 0:64],
                                    ps.rearrange("p (h d) -> p h d", d=DH))

                        # q/k projections + RoPE
                        with tc.tile_pool(name="qkw" + _sfx, bufs=1) as p_qkw, \
                             tc.tile_pool(name="stage_a", bufs=2) as p_sta, \
                             tc.tile_pool(name="psA" + _sfx, bufs=4, space="PSUM") as pp_a, \
                             tc.tile_pool(name="psR" + _sfx, bufs=2, space="PSUM") as pp_r:
                            wq_r = [p_qkw.tile([128, CG], F16, name=f"wq{c}") for c in range(KC)]
                            wk_r = [p_qkw.tile([128, CG], F16, name=f"wk{c}") for c in range(KC)]
                            for c in range(KC):
                                rsl = slice(128 * (c % 2), 128 * (c % 2) + 128)
                                nc.sync.dma_start(out=wq_r[c], in_=wq_v[c // 2, rsl])
                                nc.sync.dma_start(out=wk_r[c], in_=wkv_v[c // 2, rsl, 0:CG])
                            cosf = p_qkw.tile([128, N], F16, name="cosf")
                            nc.sync.dma_start(out=cosf[0:64], in_=cs_v[0])
                            nc.sync.dma_start(out=cosf[64:128], in_=cs_v[1])
                            sinf = p_qkw.tile([128, N], F16, name="sinf")
                            nc.sync.dma_start(out=sinf[0:64], in_=cs_v[2])
                            nc.sync.dma_start(out=sinf[64:128], in_=cs_v[3])

                            def _finish_rope(pend):
                                dst, m, n, qsb, lbl = pend
                                ns = slice(n * 512, (n + 1) * 512)
                                pr = pp_r.tile([128, 512], F32, name=f"pr{lbl}{m}{n}", tag="pr")
                                nc.tensor.matmul(pr, rt_s, qsb, start=True, stop=True)
                                t1 = p_sta.tile([128, 512], F16, name=f"t1{lbl}{m}{n}",
                                                tag="t1", bufs=2)
                                nc.vector.tensor_mul(t1, qsb, cosf[:, ns])
                                t2 = p_sta.tile([128, 512], F16, name=f"t2{lbl}{m}{n}",
                                                tag="t2", bufs=2)
                                nc.vector.tensor_mul(t2, pr, sinf[:, ns])
                                if lbl == "q":
                                    nc.vector.tensor_add(dst[m][:, ns], t1, t2)
                                else:
                                    nc.vector.tensor_add(
                                        kTz[2 * m][0:64, ns], t1[0:64], t2[0:64])
                                    nc.vector.tensor_add(
                                        kTz[2 * m + 1][64:128, ns], t1[64:128], t2[64:128])

                            pend = None
                            for m in range(MT):
                                for lbl, w_r, dst in (("q", wq_r, qT), ("k", wk_r, None)):
                                    for n in range(NQ):
                                        ns = slice(n * 512, (n + 1) * 512)
                                        ps = pp_a.tile([128, 512], F32,
                                                       name=f"ps{lbl}{m}{n}", tag="ps")
                                        for c in range(KC):
                                            nc.tensor.matmul(
                                                ps, w_r[c][:, m * 128:(m + 1) * 128],
                                                xT[c][:, ns],
                                                start=(c == 0), stop=(c == KC - 1))
                                        qsb = p_sta.tile([128, 512], F16,
                                                         name=f"qsb{lbl}{m}{n}",
                                                         tag="qsb", bufs=3)
                                        nc.vector.tensor_copy(qsb, ps)
                                        if pend is not None:
                                            _finish_rope(pend)
                                        pend = (dst, m, n, qsb, lbl)
                            _finish_rope(pend)

                    if mode == "proj":
                        with tc.tile_pool(name="dbg" + _sfx, bufs=2) as p_dbg:
                            for m in range(MT):
                                for lbl, dst_e in (("q", dq_e), ("k", dk_e)):
                                    db = p_dbg.tile([128, N], F32, name=f"db{lbl}{m}", tag="db")
                                    if lbl == "q":
                                        nc.vector.tensor_copy(db, qT[m])
                                    else:
                                        nc.vector.tensor_copy(db[0:64], kTz[2 * m][0:64])
                                        nc.vector.tensor_copy(db[64:128], kTz[2 * m + 1][64:128])
                                    nc.sync.dma_start(
                                        out=dst_e.ap().rearrange("(m p) n -> m p n", p=128)[m],
                                        in_=db)
                            dv3 = dv_e.ap().rearrange("(s p) c -> s p c", p=128)
                            for s in range(NKT):
                                vxf = p_dbg.tile([128, HG * 128], F32, name=f"dvf{s}", tag="dvf")
                                nc.vector.tensor_copy(
                                    vxf, vsb[s].rearrange("p h d -> p (h d)"))
                                nc.sync.dma_start(out=dv3[s], in_=vxf)
                        attn_enabled = False
                    else:
                        attn_enabled = True

                    # ---------- attention + output projection ----------
                    if attn_enabled:
                      with tc.tile_pool(name="oTp" + _sfx, bufs=1) as p_oT, \
                         tc.tile_pool(name="attn" + _sfx, bufs=1) as p_at, \
                         tc.tile_pool(name="wop" + _sfx, bufs=1) as p_wo, \
                         tc.tile_pool(name="psS" + _sfx, bufs=2, space="PSUM") as pp_s, \
                         tc.tile_pool(name="psO" + _sfx, bufs=4, space="PSUM") as pp_o:
                          oT = [p_oT.tile([128, N], F16, name=f"oT{m}") for m in range(MT)]
                          wo_r = [p_wo.tile([128, C], F16, name=f"wo{c}") for c in range(MT)]
                          for c in range(MT):
                              nc.sync.dma_start(out=wo_r[c], in_=wo_v[c])
                          # bias replicated across partitions once (PE outer
                          # product), so outproj adds it on the DVE copy
                          bias128 = p_wo.tile([128, C], F16, name="bias128")
                          for half in range(2):
                              osl = slice(half * 512, (half + 1) * 512)
                              ps_b = pp_o.tile([128, 512], F32,
                                               name=f"psb{half}", tag="pso")
                              nc.tensor.matmul(ps_b, ones1_r, bias_r[:, osl],
                                               start=True, stop=True)
                              nc.vector.tensor_copy(bias128[:, osl], ps_b)

                          # exp groups: 16 kc chunks -> 8 pairs (fits 4 PSUM
                          # banks for sim, leaving 4 for pso/outproj slack)
                          GRP = [(2 * i, 2 * i + 2) for i in range(8)]

                          def _emit_pv(pend_pv, pso, hp):
                              (k0, k1), exs = pend_pv
                              for half in range(2):
                                  h = hp * 2 + half
                                  for j in range(k1 - k0):
                                      kc = k0 + j
                                      nc.tensor.matmul(
                                          pso[half], vsb[kc][:, h, :], exs[half][:, j],
                                          start=(kc == 0), stop=(kc == NKT - 1))

                          def _emit_outproj(s):
                              for half in range(2):
                                  osl = slice(half * 512, (half + 1) * 512)
                                  ps = pp_o.tile([128, 512], F32, name=f"po{s}{half}",
                                                 tag="pso")
                                  for cc in range(MT):
                                      nc.tensor.matmul(
                                          ps, oT[cc][:, s * 128:(s + 1) * 128],
                                          wo_r[cc][:, osl],
                                          start=(cc == 0), stop=(cc == MT - 1))
                                  ob = p_at.tile([128, 512], F16, name=f"ob{s}{half}",
                                                 tag="ob", bufs=4)
                                  nc.vector.tensor_add(ob, ps, bias128[:, osl])
                                  nc.sync.dma_start(out=part3[s][:, osl], in_=ob)
                              if s % 4 == 3 and mode == "full":
                                  q = s // 4
                                  nc.gpsimd.collective_compute(
                                      "ReduceScatter", mybir.AluOpType.add,
                                      replica_groups=PAIRS,
                                      ins=[part_d[q * 512:(q + 1) * 512]],
                                      outs=[rs_ch[q][:]])
                                  nc.sync.dma_start(
                                      out=out_e.ap()[q * 256:(q + 1) * 256],
                                      in_=rs_ch[q][:])

                          for qc in range(NQ):
                              qs = slice(qc * 512, (qc + 1) * 512)
                              for hp in range(MT):
                                  pso = [
                                      pp_o.tile([128, 512], F32, name=f"pso{qc}{hp}{h}",
                                                tag="pso")
                                      for h in range(2)
                                  ]
                                  pend_pv = None
                                  for (k0, k1) in GRP:
                                      exs = []
                                      for half in range(2):
                                          sim = pp_s.tile([128, 2, 512], F32,
                                                          name=f"sim{qc}{hp}{k0}{half}",
                                                          tag="sim")
                                          for j in range(k1 - k0):
                                              kc = k0 + j
                                              nc.tensor.matmul(
                                                  sim[:, j],
                                                  kTz[2 * hp + half][:, kc * 128:(kc + 1) * 128],
                                                  qT[hp][:, qs],
                                                  start=True, stop=True)
                                          ex = p_at.tile([128, 2, 512], F16,
                                                         name=f"ex{qc}{hp}{k0}{half}",
                                                         tag="ex", bufs=6)
                                          nc.scalar.activation(
                                              ex[:, 0:k1 - k0], sim[:, 0:k1 - k0],
                                              EXP, scale=SCALE)
                                          exs.append(ex)
                                      if pend_pv is not None:
                                          _emit_pv(pend_pv, pso, hp)
                                      pend_pv = ((k0, k1), exs)
                                  _emit_pv(pend_pv, pso, hp)

                                  for half in range(2):
                                      rc = p_at.tile([64, 512], F32,
                                                     name=f"rc{qc}{hp}{half}", tag="rc", bufs=4)
                                      nc.vector.reciprocal(rc, pso[half][64:128])
                                      nc.vector.tensor_mul(
                                          oT[hp][half * 64:(half + 1) * 64, qs],
                                          pso[half][0:64], rc)

                              # all head pairs done for this qc: project + reduce
                              for s in range(qc * 4, qc * 4 + 4):
                                  _emit_outproj(s)
                          if mode != "full":
                              nc.sync.dma_start(out=part_e.ap(), in_=part_d[:])

    nc.compile()
    return nc


_NC = {}


def _get_nc(mode="full", reps=1):
    key = (mode, reps)
    if key not in _NC:
        _NC[key] = _build(mode, reps)
    return _NC[key]


def _rope_tables():
    inv = (1.0 / (ROPE_BASE ** (np.arange(0, DH, 2, dtype=np.float32) / DH))).astype(np.float32)
    t = np.arange(N, dtype=np.float32)
    freqs = np.outer(t, inv).astype(np.float32)           # [N, 32]
    emb = np.concatenate([freqs, freqs], axis=-1)         # [N, 64]
    cosT = np.cos(emb).astype(np.float32).T               # [64, N]
    sinT = np.sin(emb).astype(np.float32).T
    cosF = np.ascontiguousarray(np.tile(cosT, (2, 1)))    # [128, N]
    sinF = np.ascontiguousarray(np.tile(sinT, (2, 1)))
    return cosF, sinF


def _rot_matrix():
    # rotate_half as a left-multiply in [d, n] layout: rot = R @ q
    R = np.zeros((DH, DH), np.float32)
    half = DH // 2
    for d in range(half):
        R[d, d + half] = -1.0
        R[d + half, d] = 1.0
    Rbig = np.zeros((128, 128), np.float32)
    Rbig[:DH, :DH] = R
    Rbig[DH:, DH:] = R
    return np.ascontiguousarray(Rbig.T)  # lhsT for out = Rbig @ rhs


_CONST = None


def _consts():
    global _CONST
    if _CONST is None:
        cosF, sinF = _rope_tables()
        CS = np.concatenate([cosF, sinF], axis=0).astype(np.float16)  # [256, N]
        rt16 = _rot_matrix().astype(np.float16)
        _CONST = (CS, rt16)
    return _CONST


def kernel(x, Wq, Wkv, Wout, b_out):
    f16 = np.float16
    x = np.asarray(x)
    Wq = np.asarray(Wq)
    Wkv = np.asarray(Wkv)
    Wout = np.asarray(Wout)
    b_out = np.asarray(b_out)

    CS, rt16 = _consts()
    bias16 = (b_out.astype(np.float32) * 0.5).reshape(1, C).astype(f16)

    in_maps = []
    for core in range(NCORES):
        b, g = core // 2, core % 2
        gs = slice(CG * g, CG * (g + 1))
        vs = slice(C + CG * g, C + CG * (g + 1))
        rs = slice(256 * b, 256 * (b + 1))
        wkv4 = np.empty((1, PK_WKV), f16)
        pkv = wkv4.reshape(256, 2 * CG)
        pkv[:, 0:CG] = Wkv[rs, gs].astype(f16)
        pkv[:, CG:2 * CG] = Wkv[rs, vs].astype(f16)
        pack = np.empty((1, PK), f16)
        pack[0, PK_OFF[0]:PK_OFF[0] + PK_WQ] = Wq[rs, gs].astype(f16).reshape(-1)
        pack[0, PK_OFF[1]:PK_OFF[1] + PK_WO] = \
            Wout[CG * g + 128 * b:CG * g + 128 * (b + 1), :].astype(f16).reshape(-1)
        pack[0, PK_OFF[2]:PK_OFF[2] + PK_CS] = CS[64 * b:64 * (b + 1)].reshape(-1)
        in_maps.append({
            "xh": x[b, N // 2 * g:N // 2 * (g + 1)].T.astype(f16),
            "wkv4": wkv4,
            "pack": pack,
            "rt": rt16,
            "bias": bias16,
        })

    res = run_bass_kernel_spmd(_get_nc(), in_maps, core_ids=list(range(NCORES)))
    out = np.empty((B, N, C), np.float32)
    for b in range(B):
        e = res.results[2 * b]["out"].astype(np.float32)
        o = res.results[2 * b + 1]["out"].astype(np.float32)
        for q in range(4):
            out[b, 512 * q:512 * q + 256] = e[256 * q:256 * (q + 1)]
            out[b, 512 * q + 256:512 * (q + 1)] = o[256 * q:256 * (q + 1)]
    return out


# revision 8
# speedup vs baseline: 1.0401x; 1.0111x over previous
"""Multi-head self-attention with RoPE on 8 Trainium2 NeuronCores.

Transfer-minimal fp16 design.  Core c (b=c//2, g=c%2) handles batch b and
heads [8g, 8g+8).  The host sends each input byte exactly once (fp16):
  - xh   [1024,1024]  x[b] rows g*1024..+1024, PRE-TRANSPOSED to [C, N/2]
  - wkv4 [1, 262144]  flat [K-slice | V-slice], Wkv rows 256b..+256 of the
                      g-half columns (own AllGather so V-proj starts early)
  - pack [1, 393216]  flat wq (rows 256b, cols 512g) | wo (rows 512g+128b)
                      | cs (1/4 of the stacked cos/sin tables)
  - rt, bias          rotate-half permutation matrix, b_out/2
On device: a pair AllGather rebuilds x[b]^T (static placement), two
group-of-4 AllGathers (cores sharing g) rebuild the g-half weight slices
and tables at static addresses.  V (fused [v_h | ones]) stays in SBUF.
Sim matmuls use per-head zero-padded K tiles (kTz) so the contraction is
a full 128 rows.  All matmuls fp16 with fp32 PSUM accumulation; exp on
the scalar engine feeds a pipelined P@V whose ones-columns produce the
softmax denominators.  Output projection interleaves per query-chunk with
pair ReduceScatters (fp16); each core returns 1024 rows (256-row
interleave) that the host reassembles and upcasts to fp32.
"""

import numpy as np

import concourse.mybir as mybir
import concourse.tile as tile
from concourse import bacc
from concourse.bass_utils import run_bass_kernel_spmd

B, N, H, DH = 4, 2048, 16, 64
C = H * DH            # 1024
HG = H // 2           # 8 heads per core
CG = HG * DH          # 512 channels per core
NCORES = 8
ROPE_BASE = 10000.0

F16 = mybir.dt.float16
F32 = mybir.dt.float32

KC = C // 128         # 8 contraction chunks over C
MT = CG // 128        # 4 m-tiles of per-core q/k channels (2 heads each)
NQ = N // 512         # 4 query column chunks
NKT = N // 128        # 16 key/seq row tiles

PAIRS = [[0, 1], [2, 3], [4, 5], [6, 7]]
GGRP = [[0, 2, 4, 6], [1, 3, 5, 7]]
EXP = mybir.ActivationFunctionType.Exp
SCALE = float(1.0 / np.sqrt(DH))

# weight slabs (per group-of-4 member), flat f16.  wkv is its own AG (first
# weight needed); wq | wo | cs ride a second AG.
PK_WKV = 256 * 2 * CG       # 262144
PK_WQ = 256 * CG            # 131072
PK_WO = 128 * C             # 131072
PK_CS = 64 * N              # 131072
PK = PK_WQ + PK_WO + PK_CS
PK_OFF = (0, PK_WQ, PK_WQ + PK_WO)


def _build(mode="full", reps=1):
    """mode: "full" | "noccl" (skip output RS, dump partial) | "proj"
    (projections only, dump qT/kT/v)."""
    nc = bacc.Bacc("TRN2", target_bir_lowering=False, num_devices=NCORES)

    xh_e = nc.declare_dram_parameter("xh", [N // 2, C], F16, isOutput=False)
    wkv_e = nc.declare_dram_parameter("wkv4", [1, PK_WKV], F16, isOutput=False)
    pk_e = nc.declare_dram_parameter("pack", [1, PK], F16, isOutput=False)
    rt_e = nc.declare_dram_parameter("rt", [128, 128], F16, isOutput=False)
    bias_e = nc.declare_dram_parameter("bias", [1, C], F16, isOutput=False)
    if mode == "full":
        out_e = nc.declare_dram_parameter("out", [N // 2, C], F16, isOutput=True)
    elif mode == "noccl":
        part_e = nc.declare_dram_parameter("part", [N, C], F16, isOutput=True)
    elif mode == "proj":
        dq_e = nc.declare_dram_parameter("dbg_q", [CG, N], F32, isOutput=True)
        dk_e = nc.declare_dram_parameter("dbg_k", [CG, N], F32, isOutput=True)
        dv_e = nc.declare_dram_parameter("dbg_v", [NKT * 128, HG * 128], F32, isOutput=True)

    with tile.TileContext(nc) as tc:
        for _rep in range(reps):
            _sfx = f"_{_rep}" if reps > 1 else ""
            with tc.tile_pool(name="persist" + _sfx, bufs=1) as p_pers, \
                 tc.tile_pool(name="dram" + _sfx, bufs=1, space="DRAM") as p_dram:
                # ---------- DRAM scratch + collectives ----------
                wkv_d0 = p_dram.tile([4, PK_WKV], F16, name="wkv_d")
                xg_d = p_dram.tile([N, C], F16, name="xg_d")
                pk_d = p_dram.tile([4, PK], F16, name="pk_d")
                part_d = p_dram.tile([N, C], F16, name="part_d")
                part3 = part_d.rearrange("(s p) c -> s p c", p=128)
                rs_ch = [p_dram.tile([N // 8, C], F16, name=f"rs{q}")
                         for q in range(4)]

                # collectives may not read IO tensors: stage d2d first
                xh_s = p_dram.tile([N // 2, C], F16, name="xh_s")
                nc.sync.dma_start(out=xh_s, in_=xh_e.ap())
                wkv_s = p_dram.tile([1, PK_WKV], F16, name="wkv_s")
                nc.sync.dma_start(out=wkv_s, in_=wkv_e.ap())
                pk_s = p_dram.tile([1, PK], F16, name="pk_s")
                nc.sync.dma_start(out=pk_s, in_=pk_e.ap())
                wkv_d = wkv_d0
                # x AG split in two C-row halves: the first unblocks V-proj
                # accumulation over kc 0..3 while the rest still transfers
                xgA = p_dram.tile([N // 2, C], F16, name="xgA")
                xgB = p_dram.tile([N // 2, C], F16, name="xgB")
                nc.gpsimd.collective_compute(
                    "AllGather", mybir.AluOpType.bypass, replica_groups=PAIRS,
                    ins=[xh_s[0:512]], outs=[xgA[:]])
                nc.gpsimd.collective_compute(
                    "AllGather", mybir.AluOpType.bypass, replica_groups=GGRP,
                    ins=[wkv_s[:]], outs=[wkv_d[:]])
                nc.gpsimd.collective_compute(
                    "AllGather", mybir.AluOpType.bypass, replica_groups=PAIRS,
                    ins=[xh_s[512:1024]], outs=[xgB[:]])
                nc.gpsimd.collective_compute(
                    "AllGather", mybir.AluOpType.bypass, replica_groups=GGRP,
                    ins=[pk_s[:]], outs=[pk_d[:]])

                # per-rank views into the gathered slabs
                wkv_v = wkv_d.rearrange("b (r m) -> b r m", m=2 * CG)  # [4,256,1024]
                wq_v = pk_d[:, PK_OFF[0]:PK_OFF[0] + PK_WQ].rearrange(
                    "b (r m) -> b r m", m=CG)          # [4, 256, 512]
                wo_v = pk_d[:, PK_OFF[1]:PK_OFF[1] + PK_WO].rearrange(
                    "b (r m) -> b r m", m=C)           # [4, 128, 1024]
                cs_v = pk_d[:, PK_OFF[2]:PK_OFF[2] + PK_CS].rearrange(
                    "b (r m) -> b r m", m=N)           # [4, 64, 2048]

                # ---------- small constants ----------
                ones1_r = p_pers.tile([1, 128], F16, name="ones1_r")
                nc.vector.memset(ones1_r, 1.0)
                rt_s = p_pers.tile([128, 128], F16, name="rt_s")
                nc.sync.dma_start(out=rt_s, in_=rt_e.ap())
                bias_r = p_pers.tile([1, C], F16, name="bias_r")
                nc.sync.dma_start(out=bias_r, in_=bias_e.ap())

                # V (+ones) stationary tiles, persist through attention
                vsb = [p_pers.tile([128, HG, 128], F16, name=f"vsb{s}")
                       for s in range(NKT)]
                for s in range(NKT):
                    nc.vector.memset(vsb[s][:, :, 64:128], 1.0)

                with tc.tile_pool(name="qk" + _sfx, bufs=1) as p_qk:
                    qT = [p_qk.tile([128, N], F16, name=f"qT{m}") for m in range(MT)]
                    # per-head K tiles, zero-padded on the complementary dh
                    # rows so sim matmuls run full 128-row contraction:
                    # kTz[2m]   = [k_head(0:64) | 0]   (even head of pair m)
                    # kTz[2m+1] = [0 | k_head(64:128)] (odd head stays put)
                    kTz = [p_qk.tile([128, N], F16, name=f"kTz{h}") for h in range(H // 2)]
                    for m in range(MT):
                        nc.vector.memset(kTz[2 * m][64:128, :], 0.0)
                        nc.vector.memset(kTz[2 * m + 1][0:64, :], 0.0)

                    # ---------- xT + projections ----------
                    # xh arrives pre-transposed [C, N/2]; the pair AG stacks
                    # [own-half-T | pair-half-T] so xT tiles are plain loads.
                    with tc.tile_pool(name="xt" + _sfx, bufs=1) as p_xt:
                        xT = [p_xt.tile([128, N], F16, name=f"xT{c}") for c in range(KC)]
                        # [2 half, 8 cb, 128 p, 1024 n] view of the gathered x;
                        # loads ride the Activation hwdge queue (idle here) so
                        # they overlap the wv loads on the SP queue.
                        xgA_v = xgA.rearrange("(h q p) n -> q p h n", h=2, p=128)
                        xgB_v = xgB.rearrange("(h q p) n -> q p h n", h=2, p=128)
                        for cb in range(KC):
                            src = xgA_v[cb] if cb < 4 else xgB_v[cb - 4]
                            nc.scalar.dma_start(
                                out=xT[cb].rearrange("p (h n) -> p h n", h=2),
                                in_=src)

                        with tc.tile_pool(name="wvp" + _sfx, bufs=1) as p_wv, \
                             tc.tile_pool(name="psV" + _sfx, bufs=4, space="PSUM") as pp_v:
                            wv_r = [p_wv.tile([128, CG], F16, name=f"wv{c}") for c in range(KC)]
                            for c in range(KC):
                                rsl = slice(128 * (c % 2), 128 * (c % 2) + 128)
                                nc.sync.dma_start(
                                    out=wv_r[c], in_=wkv_v[c // 2, rsl, CG:2 * CG])
                            for s in range(NKT):
                                ps = pp_v.tile([128, CG], F32, name=f"pv{s}", tag="pv")
                                for c in range(KC):
                                    nc.tensor.matmul(
                                        ps, xT[c][:, s * 128:(s + 1) * 128], wv_r[c],
                                        start=(c == 0), stop=(c == KC - 1))
                                nc.vector.tensor_copy(
                                    vsb[s][:, :, 0:64],
                                    ps.rearrange("p (h d) -> p h d", d=DH))

                        # q/k projections + RoPE
                        with tc.tile_pool(name="qkw" + _sfx, bufs=1) as p_qkw, \
                             tc.tile_pool(name="stage_a", bufs=2) as p_sta, \
                             tc.tile_pool(name="psA" + _sfx, bufs=4, space="PSUM") as pp_a, \
                             tc.tile_pool(name="psR" + _sfx, bufs=2, space="PSUM") as pp_r:
                            wq_r = [p_qkw.tile([128, CG], F16, name=f"wq{c}") for c in range(KC)]
                            wk_r = [p_qkw.tile([128, CG], F16, name=f"wk{c}") for c in range(KC)]
                            for c in range(KC):
                                rsl = slice(128 * (c % 2), 128 * (c % 2) + 128)
                                nc.sync.dma_start(out=wq_r[c], in_=wq_v[c // 2, rsl])
                                nc.sync.dma_start(out=wk_r[c], in_=wkv_v[c // 2, rsl, 0:CG])
                            cosf = p_qkw.tile([128, N], F16, name="cosf")
                            nc.sync.dma_start(out=cosf[0:64], in_=cs_v[0])
                            nc.sync.dma_start(out=cosf[64:128], in_=cs_v[1])
                            sinf = p_qkw.tile([128, N], F16, name="sinf")
                            nc.sync.dma_start(out=sinf[0:64], in_=cs_v[2])
                            nc.sync.dma_start(out=sinf[64:128], in_=cs_v[3])

                            def _finish_rope(pend):
                                dst, m, n, qsb, lbl = pend
                                ns = slice(n * 512, (n + 1) * 512)
                                pr = pp_r.tile([128, 512], F32, name=f"pr{lbl}{m}{n}", tag="pr")
                                nc.tensor.matmul(pr, rt_s, qsb, start=True, stop=True)
                                t1 = p_sta.tile([128, 512], F16, name=f"t1{lbl}{m}{n}",
                                                tag="t1", bufs=2)
                                nc.vector.tensor_mul(t1, qsb, cosf[:, ns])
                                t2 = p_sta.tile([128, 512], F16, name=f"t2{lbl}{m}{n}",
                                                tag="t2", bufs=2)
                                nc.vector.tensor_mul(t2, pr, sinf[:, ns])
                                if lbl == "q":
                                    nc.vector.tensor_add(dst[m][:, ns], t1, t2)
                                else:
                                    nc.vector.tensor_add(
                                        kTz[2 * m][0:64, ns], t1[0:64], t2[0:64])
                                    nc.vector.tensor_add(
                                        kTz[2 * m + 1][64:128, ns], t1[64:128], t2[64:128])

                            pend = None
                            for m in range(MT):
                                for lbl, w_r, dst in (("q", wq_r, qT), ("k", wk_r, None)):
                                    for n in range(NQ):
                                        ns = slice(n * 512, (n + 1) * 512)
                                        ps = pp_a.tile([128, 512], F32,
                                                       name=f"ps{lbl}{m}{n}", tag="ps")
                                        for c in range(KC):
                                            nc.tensor.matmul(
                                                ps, w_r[c][:, m * 128:(m + 1) * 128],
                                                xT[c][:, ns],
                                                start=(c == 0), stop=(c == KC - 1))
                                        qsb = p_sta.tile([128, 512], F16,
                                                         name=f"qsb{lbl}{m}{n}",
                                                         tag="qsb", bufs=3)
                                        nc.vector.tensor_copy(qsb, ps)
                                        if pend is not None:
                                            _finish_rope(pend)
                                        pend = (dst, m, n, qsb, lbl)
                            _finish_rope(pend)

                    if mode == "proj":
                        with tc.tile_pool(name="dbg" + _sfx, bufs=2) as p_dbg:
                            for m in range(MT):
                                for lbl, dst_e in (("q", dq_e), ("k", dk_e)):
                                    db = p_dbg.tile([128, N], F32, name=f"db{lbl}{m}", tag="db")
                                    if lbl == "q":
                                        nc.vector.tensor_copy(db, qT[m])
                                    else:
                                        nc.vector.tensor_copy(db[0:64], kTz[2 * m][0:64])
                                        nc.vector.tensor_copy(db[64:128], kTz[2 * m + 1][64:128])
                                    nc.sync.dma_start(
                                        out=dst_e.ap().rearrange("(m p) n -> m p n", p=128)[m],
                                        in_=db)
                            dv3 = dv_e.ap().rearrange("(s p) c -> s p c", p=128)
                            for s in range(NKT):
                                vxf = p_dbg.tile([128, HG * 128], F32, name=f"dvf{s}", tag="dvf")
                                nc.vector.tensor_copy(
                                    vxf, vsb[s].rearrange("p h d -> p (h d)"))
                                nc.sync.dma_start(out=dv3[s], in_=vxf)
                        attn_enabled = False
                    else:
                        attn_enabled = True

                    # ---------- attention + output projection ----------
                    if attn_enabled:
                      with tc.tile_pool(name="oTp" + _sfx, bufs=1) as p_oT, \
                         tc.tile_pool(name="attn" + _sfx, bufs=1) as p_at, \
                         tc.tile_pool(name="wop" + _sfx, bufs=1) as p_wo, \
                         tc.tile_pool(name="psS" + _sfx, bufs=2, space="PSUM") as pp_s, \
                         tc.tile_pool(name="psO" + _sfx, bufs=4, space="PSUM") as pp_o:
                          oT = [p_oT.tile([128, N], F16, name=f"oT{m}") for m in range(MT)]
                          wo_r = [p_wo.tile([128, C], F16, name=f"wo{c}") for c in range(MT)]
                          for c in range(MT):
                              nc.sync.dma_start(out=wo_r[c], in_=wo_v[c])
                          # bias replicated across partitions once (PE outer
                          # product), so outproj adds it on the DVE copy
                          bias128 = p_wo.tile([128, C], F16, name="bias128")
                          for half in range(2):
                              osl = slice(half * 512, (half + 1) * 512)
                              ps_b = pp_o.tile([128, 512], F32,
                                               name=f"psb{half}", tag="pso")
                              nc.tensor.matmul(ps_b, ones1_r, bias_r[:, osl],
                                               start=True, stop=True)
                              nc.vector.tensor_copy(bias128[:, osl], ps_b)

                          # exp groups: 16 kc chunks -> 8 pairs (fits 4 PSUM
                          # banks for sim, leaving 4 for pso/outproj slack)
                          GRP = [(2 * i, 2 * i + 2) for i in range(8)]

                          def _emit_pv(pend_pv, pso, hp):
                              (k0, k1), exs = pend_pv
                              for half in range(2):
                                  h = hp * 2 + half
                                  for j in range(k1 - k0):
                                      kc = k0 + j
                                      nc.tensor.matmul(
                                          pso[half], vsb[kc][:, h, :], exs[half][:, j],
                                          start=(kc == 0), stop=(kc == NKT - 1))

                          def _emit_outproj(s):
                              for half in range(2):
                                  osl = slice(half * 512, (half + 1) * 512)
                                  ps = pp_o.tile([128, 512], F32, name=f"po{s}{half}",
                                                 tag="pso")
                                  for cc in range(MT):
                                      nc.tensor.matmul(
                                          ps, oT[cc][:, s * 128:(s + 1) * 128],
                                          wo_r[cc][:, osl],
                                          start=(cc == 0), stop=(cc == MT - 1))
                                  ob = p_at.tile([128, 512], F16, name=f"ob{s}{half}",
                                                 tag="ob", bufs=6)
                                  nc.vector.tensor_add(ob, ps, bias128[:, osl])
                                  nc.sync.dma_start(out=part3[s][:, osl], in_=ob)
                              if s % 4 == 3 and mode == "full":
                                  q = s // 4
                                  nc.gpsimd.collective_compute(
                                      "ReduceScatter", mybir.AluOpType.add,
                                      replica_groups=PAIRS,
                                      ins=[part_d[q * 512:(q + 1) * 512]],
                                      outs=[rs_ch[q][:]])
                                  nc.sync.dma_start(
                                      out=out_e.ap()[q * 256:(q + 1) * 256],
                                      in_=rs_ch[q][:])

                          for qc in range(NQ):
                              qs = slice(qc * 512, (qc + 1) * 512)
                              for hp in range(MT):
                                  pso = [
                                      pp_o.tile([128, 512], F32, name=f"pso{qc}{hp}{h}",
                                                tag="pso")
                                      for h in range(2)
                                  ]
                                  pend_pv = None
                                  for (k0, k1) in GRP:
                                      exs = []
                                      for half in range(2):
                                          sim = pp_s.tile([128, 2, 512], F32,
                                                          name=f"sim{qc}{hp}{k0}{half}",
                                                          tag="sim")
                                          for j in range(k1 - k0):
                                              kc = k0 + j
                                              nc.tensor.matmul(
                                                  sim[:, j],
                                                  kTz[2 * hp + half][:, kc * 128:(kc + 1) * 128],
                                                  qT[hp][:, qs],
                                                  start=True, stop=True)
                                          ex = p_at.tile([128, 2, 512], F16,
                                                         name=f"ex{qc}{hp}{k0}{half}",
                                                         tag="ex", bufs=8)
                                          nc.scalar.activation(
                                              ex[:, 0:k1 - k0], sim[:, 0:k1 - k0],
                                              EXP, scale=SCALE)
                                          exs.append(ex)
                                      if pend_pv is not None:
                                          _emit_pv(pend_pv, pso, hp)
                                      pend_pv = ((k0, k1), exs)
                                  _emit_pv(pend_pv, pso, hp)

                                  for half in range(2):
                                      rc = p_at.tile([64, 512], F32,
                                                     name=f"rc{qc}{hp}{half}", tag="rc", bufs=6)
                                      nc.vector.reciprocal(rc, pso[half][64:128])
                                      nc.vector.tensor_mul(
                                          oT[hp][half * 64:(half + 1) * 64, qs],
                                          pso[half][0:64], rc)

                              # all head pairs done for this qc: project + reduce
                              for s in range(qc * 4, qc * 4 + 4):
                                  _emit_outproj(s)
                          if mode != "full":
                              nc.sync.dma_start(out=part_e.ap(), in_=part_d[:])

    nc.compile()
    return nc


_NC = {}


def _get_nc(mode="full", reps=1):
    key = (mode, reps)
    if key not in _NC:
        _NC[key] = _build(mode, reps)
    return _NC[key]


def _rope_tables():
    inv = (1.0 / (ROPE_BASE ** (np.arange(0, DH, 2, dtype=np.float32) / DH))).astype(np.float32)
    t = np.arange(N, dtype=np.float32)
    freqs = np.outer(t, inv).astype(np.float32)           # [N, 32]
    emb = np.concatenate([freqs, freqs], axis=-1)         # [N, 64]
    cosT = np.cos(emb).astype(np.float32).T               # [64, N]
    sinT = np.sin(emb).astype(np.float32).T
    cosF = np.ascontiguousarray(np.tile(cosT, (2, 1)))    # [128, N]
    sinF = np.ascontiguousarray(np.tile(sinT, (2, 1)))
    return cosF, sinF


def _rot_matrix():
    # rotate_half as a left-multiply in [d, n] layout: rot = R @ q
    R = np.zeros((DH, DH), np.float32)
    half = DH // 2
    for d in range(half):
        R[d, d + half] = -1.0
        R[d + half, d] = 1.0
    Rbig = np.zeros((128, 128), np.float32)
    Rbig[:DH, :DH] = R
    Rbig[DH:, DH:] = R
    return np.ascontiguousarray(Rbig.T)  # lhsT for out = Rbig @ rhs


_CONST = None


def _consts():
    global _CONST
    if _CONST is None:
        cosF, sinF = _rope_tables()
        CS = np.concatenate([cosF, sinF], axis=0).astype(np.float16)  # [256, N]
        rt16 = _rot_matrix().astype(np.float16)
        _CONST = (CS, rt16)
    return _CONST


def kernel(x, Wq, Wkv, Wout, b_out):
    f16 = np.float16
    x = np.asarray(x)
    Wq = np.asarray(Wq)
    Wkv = np.asarray(Wkv)
    Wout = np.asarray(Wout)
    b_out = np.asarray(b_out)

    CS, rt16 = _consts()
    bias16 = (b_out.astype(np.float32) * 0.5).reshape(1, C).astype(f16)

    in_maps = []
    for core in range(NCORES):
        b, g = core // 2, core % 2
        gs = slice(CG * g, CG * (g + 1))
        vs = slice(C + CG * g, C + CG * (g + 1))
        rs = slice(256 * b, 256 * (b + 1))
        wkv4 = np.empty((1, PK_WKV), f16)
        pkv = wkv4.reshape(256, 2 * CG)
        pkv[:, 0:CG] = Wkv[rs, gs].astype(f16)
        pkv[:, CG:2 * CG] = Wkv[rs, vs].astype(f16)
        pack = np.empty((1, PK), f16)
        pack[0, PK_OFF[0]:PK_OFF[0] + PK_WQ] = Wq[rs, gs].astype(f16).reshape(-1)
        pack[0, PK_OFF[1]:PK_OFF[1] + PK_WO] = \
            Wout[CG * g + 128 * b:CG * g + 128 * (b + 1), :].astype(f16).reshape(-1)
        pack[0, PK_OFF[2]:PK_OFF[2] + PK_CS] = CS[64 * b:64 * (b + 1)].reshape(-1)
        in_maps.append({
            "xh": x[b, N // 2 * g:N // 2 * (g + 1)].T.astype(f16),
            "wkv4": wkv4,
            "pack": pack,
            "rt": rt16,
            "bias": bias16,
        })

    res = run_bass_kernel_spmd(_get_nc(), in_maps, core_ids=list(range(NCORES)))
    out = np.empty((B, N, C), np.float32)
    for b in range(B):
        e = res.results[2 * b]["out"].astype(np.float32)
        o = res.results[2 * b + 1]["out"].astype(np.float32)
        for q in range(4):
            out[b, 512 * q:512 * q + 256] = e[256 * q:256 * (q + 1)]
            out[b, 512 * q + 256:512 * (q + 1)] = o[256 * q:256 * (q + 1)]
    return out
